# revision 13
# baseline (speedup 1.0000x reference)
"""Trainium2 Bass kernel for nn_FilmLayerNetwork.

Sharding: one NeuronCore per SMAB processor (NPROC = 8 = n_cores).
Each core computes its processor's full 512-map output slice.

v2 design notes (from NTFF traces of the 34.5us baseline):
- ALL weights are fp8e4m3 (tolerance is 2e-2; measured e2e error ~1e-4).
  fp8/bf16 matmuls are single-pass (~27-60ns) vs fp32 LOW/HIGH pairs
  (~160-480ns each) and LDWEIGHTS drops ~5x. DMA bytes drop ~3x.
- Four input DMAs (a: P+w1+xT, b: wa+wqr, c: fc1/wq2/wk2/wv2/fc2/wo/E,
  f: fp32 film tail data), issued from four different engines so the
  ~600ns DGE-issue sequencer costs parallelize; transfers still
  serialize on the shared DMA-engine pool in issue order a,f,b,c
  (consumption order).
- The exp activation table load (1.28us) is triggered by a dummy exp at
  program start so it overlaps the input DMA instead of sitting in the
  softmax critical path.
- Intermediates are bf16: single-pass matmuls everywhere, 2x DVE.
- Fusions: relu+residual-add via scalar_tensor_tensor(max,add);
  exp+row-sum via activation(accum_out); (a2e*v2T, sum) via
  tensor_tensor_reduce; FiLM tail algebra reduced to 2 on-path ops
  (out = (alpha*regs)*trans + B) with B = A - alpha*(A - offs),
  A = sum((De*regs | offs) * (gate|1)) precomputed off-path on gpsimd.
- Stage-1 O^T accumulates per-head via tile_position=(0,32h) matmuls
  writing disjoint PSUM partition ranges (no masked-V copies).
"""

import numpy as np

NM, ZG, HID, SEQ = 512, 512, 96, 8
H1, H2, NPROC, NB = 3, 16, 8, 2
SCL = float(1.0 / np.sqrt(96.0))

# a columns: P4 | w1 (4 x [wq1|wk1|wv1]) | xT4 | ones | ba rows (part 0)
A_P, A_W1, A_XT, A_ONE, A_BA8 = 0, 32, 1184, 1188, 1189
A_COLS = 1701
# b columns: wa (4 x 512) | wqr (4 x 96)
B_WA, B_WQR = 0, 2048
B_COLS = 2432
# c columns (rows 0-95): fc1 | wq2 | wk2 | wv2 | fc2 | wo (4 x 128) |
#   E2T (rows 0-15) | E96 | I8 (rows 0-7)
C_F1, C_WQ2, C_WK2, C_WV2, C_F2, C_WO, C_E2T, C_E96, C_I8 = (
    0, 96, 192, 288, 384, 480, 992, 1088, 1104)
C_COLS = 1112
# f columns (fp32): De'9 (4m x 9s) | gate9 | baT4 | bqr | -offs | regsT4
F_DE, F_G9, F_BA, F_BQR, F_NOF, F_RG = 0, 36, 45, 49, 50, 51
F_COLS = 55

_CACHE = {}


def _build_nc():
    import concourse.bass as bass
    import concourse.bacc as bacc
    import concourse.tile as tile
    import concourse.mybir as mybir

    f32 = mybir.dt.float32
    bf16 = mybir.dt.bfloat16
    f8 = mybir.dt.float8e4
    AX = mybir.AxisListType
    ALU = mybir.AluOpType
    ACT = mybir.ActivationFunctionType

    nc = bacc.Bacc("TRN2", target_bir_lowering=False, debug=False,
                   num_devices=NPROC)

    d_a = nc.dram_tensor("a", [128, A_COLS], f8, kind="ExternalInput").ap()
    d_b = nc.dram_tensor("b", [128, B_COLS], f8, kind="ExternalInput").ap()
    d_c = nc.dram_tensor("c", [128, C_COLS], f8, kind="ExternalInput").ap()
    d_f = nc.dram_tensor("f", [128, F_COLS], f32, kind="ExternalInput").ap()
    d_out = nc.dram_tensor("out", [128, 4], f32, kind="ExternalOutput").ap()

    with tile.TileContext(nc) as tc, \
         tc.tile_pool(name="sb", bufs=1) as sb, \
         tc.tile_pool(name="ps", bufs=8, space="PSUM") as ps:

        def sbt(shape, tag, dt=f32):
            return sb.tile(shape, dt, tag=tag, name=tag)

        def pst(shape, tag):
            return ps.tile(shape, f32, tag="ps_shared", name=tag)

        # ---- input DMAs: all issued from ACT in consumption order (multi
        # engine issue triggered a much longer framework init preamble);
        # the dummy exp (ACT table preload) is slotted right after `a` ----
        sb_a = sbt([128, A_COLS], "sb_a", f8)
        nc.scalar.dma_start(out=sb_a[:], in_=d_a[:])

        sb_z1 = sbt([1, 1], "sb_z1")
        nc.gpsimd.memset(sb_z1[:], 0.0)
        sb_t32 = sbt([32, 288], "sb_t32", bf16)
        nc.gpsimd.memset(sb_t32[:], 0.0)
        sb_a32 = sbt([32, 96], "sb_a32", bf16)
        nc.gpsimd.memset(sb_a32[:], 0.0)
        sb_z1e = sbt([1, 1], "sb_z1e")
        nc.scalar.activation(sb_z1e[:], sb_z1[:], ACT.Exp)

        sb_b = sbt([128, B_COLS], "sb_b", f8)
        nc.scalar.dma_start(out=sb_b[:], in_=d_b[:])
        sb_f = sbt([128, F_COLS], "sb_f")
        nc.scalar.dma_start(out=sb_f[:], in_=d_f[:])
        sb_c = sbt([128, C_COLS], "sb_c", f8)
        nc.scalar.dma_start(out=sb_c[:], in_=d_c[:])

        P_blk = lambda k: sb_a[:, A_P + 8 * k:A_P + 8 * k + 8]
        w1_blk = lambda k: sb_a[:, A_W1 + 288 * k:A_W1 + 288 * k + 288]
        xT_blk = lambda k: sb_a[:, A_XT + k:A_XT + k + 1]

        # ---- stage 0: [Qk | Kk | Vv] (8, 288) in 4 fused fp8 matmuls ----
        ps_qkv = pst([8, 288], "ps_qkv")
        for k in range(4):
            nc.tensor.matmul(ps_qkv[:], P_blk(k), w1_blk(k),
                             start=(k == 0), stop=(k == 3))

        # one copy into the 32-col-block transpose layout (bf16);
        # Vv stays untransposed at cols 192:288
        nc.scalar.copy(sb_t32[0:8, 0:288], ps_qkv[:])

        sb_tT = sbt([32, 192], "sb_tT", bf16)
        nc.vector.transpose(sb_tT[:], sb_t32[:, 0:192])

        def QkT_h(h):
            return sb_tT[0:32, 32 * h:32 * h + 8]

        def KkT_h(h):
            return sb_tT[0:32, 96 + 32 * h:96 + 32 * h + 8]

        def Vv_h(h):
            return sb_t32[0:8, 192 + 32 * h:192 + 32 * h + 32]

        # MHA1 scores, per head
        ps_s = pst([8, 24], "ps_s")
        for h in range(3):
            nc.tensor.matmul(ps_s[:, 8 * h:8 * h + 8], QkT_h(h), KkT_h(h))

        # softmax1 (no max-subtraction; magnitudes are small), normalized A
        # written into the 32x32-block layout
        a32v = sb_a32[0:8, :].rearrange("p (h x) -> p h x", h=3)[:, :, 0:8]
        nc.scalar.activation(a32v, ps_s[:].rearrange("p (h x) -> p h x", h=3),
                             ACT.Exp, scale=SCL)
        sb_sums = sbt([8, 3], "sb_sums")
        nc.vector.tensor_reduce(sb_sums[:], a32v, AX.X, ALU.add)
        sb_rec = sbt([8, 3], "sb_rec")
        nc.vector.reciprocal(sb_rec[:], sb_sums[:])
        rec_ap = sb_rec[:]
        rec_bc = bass.AP(tensor=rec_ap.tensor, offset=rec_ap.offset,
                         ap=[rec_ap.ap[0], rec_ap.ap[1], [0, 8]])
        nc.vector.tensor_tensor(a32v, a32v, rec_bc, ALU.mult)
        sb_aT32 = sbt([32, 96], "sb_aT32", bf16)
        nc.vector.transpose(sb_aT32[:], sb_a32[:])

        def A_T(h):
            return sb_aT32[0:8, 32 * h:32 * h + 8]

        # qT: 4 fp8 contraction chunks over Wqr (b landed by now)
        ps_qT = pst([96, 1], "ps_qT")
        for k in range(4):
            nc.tensor.matmul(ps_qT[:],
                             sb_b[:, B_WQR + 96 * k:B_WQR + 96 * k + 96],
                             xT_blk(k), start=(k == 0), stop=(k == 3))

        # alphaT (128,4): 16 fp8 (k,m) chunk matmuls + a 5th 1-row matmul
        # per column adding ba (stationary ba row on partition 0, moving a
        # const-1 column); sigmoid then reads PSUM directly
        ps_al = pst([128, 4], "ps_al")

        def alpha_mms(ms):
            for m in ms:
                for k in range(4):
                    nc.tensor.matmul(
                        ps_al[:, m:m + 1],
                        sb_b[:, 512 * k + 128 * m:512 * k + 128 * m + 128],
                        xT_blk(k), start=(k == 0), stop=False)
                nc.tensor.matmul(
                    ps_al[:, m:m + 1],
                    sb_a[0:1, A_BA8 + 128 * m:A_BA8 + 128 * m + 128],
                    sb_a[0:1, A_ONE:A_ONE + 1], start=False, stop=True)

        alpha_mms([0, 1])

        # O^T per head via tile_position: each head accumulates
        # Vv_h^T @ A_h^T plus Qk_h^T (via an identity-matmul) into its own
        # 32-row PSUM partition range -> H^T lands assembled, no copies
        ps_oT = pst([96, 8], "ps_oT")
        I8 = sb_c[0:8, C_I8:C_I8 + 8]
        for h in range(3):
            nc.tensor.matmul(ps_oT[32 * h:32 * h + 32, :], Vv_h(h), A_T(h),
                             start=True, stop=False, tile_position=(0, 32 * h))
            nc.tensor.matmul(ps_oT[32 * h:32 * h + 32, :],
                             sb_t32[0:8, 32 * h:32 * h + 32], I8,
                             start=False, stop=True, tile_position=(0, 32 * h))

        # qqT = wq2^T @ qT (qT relu'd on ACT during softmax)
        sb_qT = sbt([96, 1], "sb_qT", bf16)
        nc.scalar.activation(sb_qT[:], ps_qT[:], ACT.Relu,
                             bias=sb_f[0:96, F_BQR:F_BQR + 1])
        ps_qqT = pst([96, 1], "ps_qqT")
        nc.tensor.matmul(ps_qqT[:], sb_c[0:96, C_WQ2:C_WQ2 + 96], sb_qT[:])

        alpha_mms([2, 3])

        sb_hT = sbt([96, 8], "sb_hT", bf16)
        nc.vector.tensor_copy(sb_hT[:], ps_oT[:])

        # ---- fc1 residual: h2T = hT + relu(fc1^T @ hT), fused ----
        ps_rT = pst([96, 8], "ps_rT")
        nc.tensor.matmul(ps_rT[:], sb_c[0:96, C_F1:C_F1 + 96], sb_hT[:])
        sb_h2T = sbt([96, 8], "sb_h2T", bf16)
        nc.vector.scalar_tensor_tensor(sb_h2T[:], ps_rT[:], 0.0, sb_hT[:],
                                       ALU.max, ALU.add)

        # de/A precompute on gpsimd (only needs f): A = sum(De'9 * gate9)
        # with regs and the +offs fold baked in host-side
        sb_dp = sbt([128, 36], "sb_dp")
        de_v = sb_f[:, F_DE:F_DE + 36].rearrange("p (m s) -> p m s", m=4)
        g_ap = sb_f[:, F_G9:F_G9 + 9]
        g_bc = bass.AP(tensor=g_ap.tensor, offset=g_ap.offset,
                       ap=[g_ap.ap[0], [0, 4], g_ap.ap[1]])
        nc.gpsimd.tensor_tensor(sb_dp[:].rearrange("p (m s) -> p m s", m=4),
                                de_v, g_bc, ALU.mult)
        sb_A = sbt([128, 4], "sb_A")
        nc.vector.tensor_reduce(sb_A[:],
                                sb_dp[:].rearrange("p (m s) -> p m s", m=4),
                                AX.X, ALU.add)
        # de_r = A - offs  (F_NOF holds -offs)
        sb_der = sbt([128, 4], "sb_der")
        nc.gpsimd.tensor_scalar_add(sb_der[:], sb_A[:],
                                    sb_f[:, F_NOF:F_NOF + 1])

        # ---- stage 2 ----
        ps_k2T = pst([96, 8], "ps_k2T")
        nc.tensor.matmul(ps_k2T[:], sb_c[0:96, C_WK2:C_WK2 + 96], sb_h2T[:])
        ps_v2T = pst([96, 8], "ps_v2T")
        nc.tensor.matmul(ps_v2T[:], sb_c[0:96, C_WV2:C_WV2 + 96], sb_h2T[:])

        sb_qqT = sbt([96, 1], "sb_qqT")
        nc.vector.tensor_copy(sb_qqT[:], ps_qqT[:])
        sb_tmp = sbt([96, 8], "sb_tmp", bf16)
        nc.vector.tensor_scalar_mul(sb_tmp[:], ps_k2T[:], sb_qqT[:])

        ps_s2 = pst([16, 8], "ps_s2")
        nc.tensor.matmul(ps_s2[:], sb_c[0:96, C_E96:C_E96 + 16], sb_tmp[:])

        # alpha sigmoid tail: 1/(1+exp(-z)) straight off PSUM (ba was folded
        # into the matmul); +1 on gpsimd, reciprocal on DVE
        sb_en = sbt([128, 4], "sb_en")
        nc.scalar.activation(sb_en[:], ps_al[:], ACT.Exp, scale=-1.0)

        sb_v2T = sbt([96, 8], "sb_v2T", bf16)
        nc.scalar.copy(sb_v2T[:], ps_v2T[:])

        # softmax2: exp with fused row-sum accumulator
        sb_e2 = sbt([16, 8], "sb_e2", bf16)
        sb_sum2 = sbt([16, 1], "sb_sum2")
        nc.scalar.activation(sb_e2[:], ps_s2[:], ACT.Exp, scale=SCL,
                             accum_out=sb_sum2[:])
        sb_rec2 = sbt([16, 1], "sb_rec2")
        nc.vector.reciprocal(sb_rec2[:], sb_sum2[:])
        sb_a2 = sbt([16, 8], "sb_a2", bf16)
        nc.vector.tensor_scalar_mul(sb_a2[:], sb_e2[:], sb_rec2[:])

        ps_a2e = pst([96, 8], "ps_a2e")
        nc.tensor.matmul(ps_a2e[:], sb_c[0:16, C_E2T:C_E2T + 96], sb_a2[:])

        sb_dn = sbt([128, 4], "sb_dn")
        nc.gpsimd.tensor_scalar_add(sb_dn[:], sb_en[:], 1.0)
        sb_alp = sbt([128, 4], "sb_alp")
        nc.vector.reciprocal(sb_alp[:], sb_dn[:])
        sb_alr = sbt([128, 4], "sb_alr")
        nc.gpsimd.tensor_tensor(sb_alr[:], sb_alp[:],
                                sb_f[:, F_RG:F_RG + 4], ALU.mult)
        sb_D = sbt([128, 4], "sb_D")
        nc.gpsimd.tensor_tensor(sb_D[:], sb_alp[:], sb_der[:], ALU.mult)
        sb_B = sbt([128, 4], "sb_B")
        nc.gpsimd.tensor_tensor(sb_B[:], sb_A[:], sb_D[:], ALU.subtract)

        # O2 = sum_h A2 * V2 (broadcast via E2T matmul)
        # (tensor_tensor_reduce crashes HW - NRT_EXEC_UNIT_UNRECOVERABLE)
        sb_scr = sbt([96, 8], "sb_scr")
        sb_o2T = sbt([96, 1], "sb_o2T")
        nc.vector.tensor_mul(sb_scr[:], ps_a2e[:], sb_v2T[:])
        nc.vector.tensor_reduce(sb_o2T[:], sb_scr[:], AX.X, ALU.add)
        sb_ot1 = sbt([96, 1], "sb_ot1", bf16)
        nc.vector.tensor_add(sb_ot1[:], sb_o2T[:], sb_qqT[:])

        # fc2 residual, fused relu+add
        ps_r2 = pst([96, 1], "ps_r2")
        nc.tensor.matmul(ps_r2[:], sb_c[0:96, C_F2:C_F2 + 96], sb_ot1[:])
        sb_otf = sbt([96, 1], "sb_otf", bf16)
        nc.vector.scalar_tensor_tensor(sb_otf[:], ps_r2[:], 0.0, sb_ot1[:],
                                       ALU.max, ALU.add)

        # ---- tail: transT then 2-op FiLM mix ----
        ps_tr = pst([128, 4], "ps_tr")
        for m in range(4):
            nc.tensor.matmul(ps_tr[:, m:m + 1],
                             sb_c[0:96, C_WO + 128 * m:C_WO + 128 * m + 128],
                             sb_otf[:])
        sb_t1 = sbt([128, 4], "sb_t1")
        nc.vector.tensor_mul(sb_t1[:], ps_tr[:], sb_alr[:])
        sb_o = sbt([128, 4], "sb_o")
        nc.vector.tensor_add(sb_o[:], sb_t1[:], sb_B[:])

        nc.scalar.dma_start(out=d_out[:], in_=sb_o[:])

    nc.compile()
    return nc


def _to_chunks128(a, cols):
    """(512, cols) -> (128, 4*cols) with column block k = rows [128k, 128k+128)."""
    return np.ascontiguousarray(
        a.reshape(4, 128, cols).transpose(1, 0, 2).reshape(128, 4 * cols))


def _pack_inputs(inputs):
    import ml_dtypes
    fp8 = ml_dtypes.float8_e4m3
    f32 = np.float32

    gate = np.asarray(inputs['gate'], f32)
    x = np.asarray(inputs['x'], f32)
    Wa = np.asarray(inputs['Wa'], f32)
    ba = np.asarray(inputs['ba'], f32)
    Wqr = np.asarray(inputs['Wqr'], f32)
    bqr = np.asarray(inputs['bqr'], f32)
    P = np.asarray(inputs['P'], f32)
    De = np.asarray(inputs['De'], f32)
    regs = np.asarray(inputs['regs'], f32)
    Wq1 = np.asarray(inputs['Wq1'], f32)
    Wk1 = np.asarray(inputs['Wk1'], f32)
    Wv1 = np.asarray(inputs['Wv1'], f32)
    fc1 = np.asarray(inputs['fc1'], f32)
    Wq2 = np.asarray(inputs['Wq2'], f32)
    Wk2 = np.asarray(inputs['Wk2'], f32)
    Wv2 = np.asarray(inputs['Wv2'], f32)
    fc2 = np.asarray(inputs['fc2'], f32)
    Wo = np.asarray(inputs['Wo'], f32)

    # b: wa chunks then wqr chunks (shared across cores)
    b_pack = np.concatenate([
        _to_chunks128(Wa, NM),
        _to_chunks128(Wqr, HID),
    ], axis=1).astype(fp8)
    b_pack = np.ascontiguousarray(b_pack)

    xT4 = np.ascontiguousarray(x.reshape(4, 128).T)
    baT4 = np.ascontiguousarray(ba.reshape(4, 128).T)
    g9 = np.concatenate([gate.reshape(1, 8), [[1.0]]], axis=1)
    g9_128 = np.ascontiguousarray(np.tile(g9, (128, 1)), f32)

    # E masks for 16 heads of dim 6
    idx = np.arange(HID) // 6
    E96 = np.zeros((96, 16), f32)
    E96[np.arange(96), idx] = 1.0
    E2T = np.ascontiguousarray(E96.T)         # (16, 96)

    in_maps = []
    for i in range(NPROC):
        bi, t = i // 4, i % 4
        offs = 1.0 if t in (0, 2) else 0.0

        w1 = np.concatenate(
            [np.concatenate([Wq1[i][128 * k:128 * k + 128],
                             Wk1[i][128 * k:128 * k + 128],
                             Wv1[i][128 * k:128 * k + 128]], axis=1)
             for k in range(4)], axis=1)
        ones_ba = np.zeros((128, 1 + NM), f32)
        ones_ba[:, 0] = 1.0
        ones_ba[0, 1:] = ba
        a_pack = np.ascontiguousarray(np.concatenate([
            _to_chunks128(P[bi, t], SEQ), w1, xT4, ones_ba],
            axis=1).astype(fp8))

        c_np = np.zeros((128, C_COLS), f32)
        c_np[0:96, C_F1:C_F1 + 96] = fc1[i]
        c_np[0:96, C_WQ2:C_WQ2 + 96] = Wq2[i]
        c_np[0:96, C_WK2:C_WK2 + 96] = Wk2[i]
        c_np[0:96, C_WV2:C_WV2 + 96] = Wv2[i]
        c_np[0:96, C_F2:C_F2 + 96] = fc2[i]
        c_np[0:96, C_WO:C_WO + 512] = Wo[i]
        c_np[0:16, C_E2T:C_E2T + 96] = E2T
        c_np[0:96, C_E96:C_E96 + 16] = E96
        c_np[0:8, C_I8:C_I8 + 8] = np.eye(8, dtype=f32)
        c_pack = np.ascontiguousarray(c_np.astype(fp8))

        de9 = np.zeros((128, 4, 9), f32)
        de9[:, :, 0:8] = (De[bi, t] * regs[bi, t][:, None]).reshape(
            4, 128, SEQ).transpose(1, 0, 2)
        de9[:, :, 8] = offs
        f_np = np.zeros((128, F_COLS), f32)
        f_np[:, F_DE:F_DE + 36] = de9.reshape(128, 36)
        f_np[:, F_G9:F_G9 + 9] = g9_128
        f_np[:, F_BA:F_BA + 4] = baT4
        f_np[0:96, F_BQR] = bqr
        f_np[:, F_NOF] = -offs
        f_np[:, F_RG:F_RG + 4] = regs[bi, t].reshape(4, 128).T

        in_maps.append({
            'a': a_pack,
            'b': b_pack,
            'c': c_pack,
            'f': np.ascontiguousarray(f_np),
        })
    return in_maps


def _run(inputs, trace=False):
    from concourse.bass_utils import run_bass_kernel_spmd
    if 'nc' not in _CACHE:
        _CACHE['nc'] = _build_nc()
    nc = _CACHE['nc']
    in_maps = _pack_inputs(inputs)
    res = run_bass_kernel_spmd(nc, in_maps, list(range(NPROC)), trace=trace)
    out = np.zeros((NB, 4, NM), np.float32)
    for i in range(NPROC):
        out[i // 4, i % 4] = np.asarray(res.results[i]['out']).T.reshape(NM)
    return out, res


def kernel(**inputs):
    out, _ = _run(inputs, trace=False)
    return out


# revision 22
# speedup vs baseline: 1.0309x; 1.0309x over previous
"""Trainium2 Bass kernel for nn_FilmLayerNetwork.

Sharding: one NeuronCore per SMAB processor (NPROC = 8 = n_cores).
Each core computes its processor's full 512-map output slice.

v2 design notes (from NTFF traces of the 34.5us baseline):
- ALL weights are fp8e4m3 (tolerance is 2e-2; measured e2e error ~1e-4).
  fp8/bf16 matmuls are single-pass (~27-60ns) vs fp32 LOW/HIGH pairs
  (~160-480ns each) and LDWEIGHTS drops ~5x. DMA bytes drop ~3x.
- Four input DMAs (a: P+w1+xT, b: wa+wqr, c: fc1/wq2/wk2/wv2/fc2/wo/E,
  f: fp32 film tail data), issued from four different engines so the
  ~600ns DGE-issue sequencer costs parallelize; transfers still
  serialize on the shared DMA-engine pool in issue order a,f,b,c
  (consumption order).
- The exp activation table load (1.28us) is triggered by a dummy exp at
  program start so it overlaps the input DMA instead of sitting in the
  softmax critical path.
- Intermediates are bf16: single-pass matmuls everywhere, 2x DVE.
- Fusions: relu+residual-add via scalar_tensor_tensor(max,add);
  exp+row-sum via activation(accum_out); (a2e*v2T, sum) via
  tensor_tensor_reduce; FiLM tail algebra reduced to 2 on-path ops
  (out = (alpha*regs)*trans + B) with B = A - alpha*(A - offs),
  A = sum((De*regs | offs) * (gate|1)) precomputed off-path on gpsimd.
- Stage-1 O^T accumulates per-head via tile_position=(0,32h) matmuls
  writing disjoint PSUM partition ranges (no masked-V copies).
"""

import numpy as np

NM, ZG, HID, SEQ = 512, 512, 96, 8
H1, H2, NPROC, NB = 3, 16, 8, 2
SCL = float(1.0 / np.sqrt(96.0))

# a columns: P4 | w1 (4 x [wq1|wk1|wv1]) | xT4
A_P, A_W1, A_XT = 0, 32, 1184
A_COLS = 1188
# b columns: wa (4 x 512) | wqr (4 x 96)
B_WA, B_WQR = 0, 2048
B_COLS = 2432
# c columns (rows 0-95): fc1 | wq2 | wk2 | wv2 | fc2 | wo (4 x 128) |
#   E2T (rows 0-15) | E96 | I8 (rows 0-7)
C_F1, C_WQ2, C_WK2, C_WV2, C_F2, C_WO, C_E2T, C_E96, C_I8 = (
    0, 96, 192, 288, 384, 480, 992, 1088, 1104)
C_COLS = 1112
# f columns (fp32): De'9 (4m x 9s) | gate9 | exp(-baT) | bqr | -offs | regsT4
F_DE, F_G9, F_ENB, F_BQR, F_NOF, F_RG = 0, 36, 45, 49, 50, 51
F_COLS = 55

_CACHE = {}


def _build_nc():
    import concourse.bass as bass
    import concourse.bacc as bacc
    import concourse.tile as tile
    import concourse.mybir as mybir

    f32 = mybir.dt.float32
    bf16 = mybir.dt.bfloat16
    f8 = mybir.dt.float8e4
    AX = mybir.AxisListType
    ALU = mybir.AluOpType
    ACT = mybir.ActivationFunctionType

    nc = bacc.Bacc("TRN2", target_bir_lowering=False, debug=False,
                   num_devices=NPROC)

    d_a = nc.dram_tensor("a", [128, A_COLS], f8, kind="ExternalInput").ap()
    d_b = nc.dram_tensor("b", [128, B_COLS], f8, kind="ExternalInput").ap()
    d_c = nc.dram_tensor("c", [128, C_COLS], f8, kind="ExternalInput").ap()
    d_f = nc.dram_tensor("f", [128, F_COLS], f32, kind="ExternalInput").ap()
    d_out = nc.dram_tensor("out", [128, 4], f32, kind="ExternalOutput").ap()

    with tile.TileContext(nc) as tc, \
         tc.tile_pool(name="sb", bufs=1) as sb, \
         tc.tile_pool(name="ps", bufs=8, space="PSUM") as ps:

        def sbt(shape, tag, dt=f32):
            return sb.tile(shape, dt, tag=tag, name=tag)

        def pst(shape, tag):
            return ps.tile(shape, f32, tag="ps_shared", name=tag)

        # ---- input DMAs: all issued from ACT in consumption order (multi
        # engine issue triggered a much longer framework init preamble);
        # the dummy exp (ACT table preload) is slotted right after `a` ----
        sb_a = sbt([128, A_COLS], "sb_a", f8)
        nc.scalar.dma_start(out=sb_a[:], in_=d_a[:])

        sb_z1 = sbt([1, 1], "sb_z1")
        nc.gpsimd.memset(sb_z1[:], 0.0)
        sb_t32 = sbt([32, 288], "sb_t32", bf16)
        nc.gpsimd.memset(sb_t32[:], 0.0)
        sb_a32 = sbt([32, 96], "sb_a32", bf16)
        nc.gpsimd.memset(sb_a32[:], 0.0)
        sb_z1e = sbt([1, 1], "sb_z1e")
        nc.scalar.activation(sb_z1e[:], sb_z1[:], ACT.Exp)

        sb_b = sbt([128, B_COLS], "sb_b", f8)
        nc.scalar.dma_start(out=sb_b[:], in_=d_b[:])
        sb_f = sbt([128, F_COLS], "sb_f")
        nc.scalar.dma_start(out=sb_f[:], in_=d_f[:])
        sb_c = sbt([128, C_COLS], "sb_c", f8)
        nc.scalar.dma_start(out=sb_c[:], in_=d_c[:])

        P_blk = lambda k: sb_a[:, A_P + 8 * k:A_P + 8 * k + 8]
        w1_blk = lambda k: sb_a[:, A_W1 + 288 * k:A_W1 + 288 * k + 288]
        xT_blk = lambda k: sb_a[:, A_XT + k:A_XT + k + 1]

        # ---- stage 0: [Qk | Kk | Vv] (8, 288) in 4 fused fp8 matmuls ----
        ps_qkv = pst([8, 288], "ps_qkv")
        for k in range(4):
            nc.tensor.matmul(ps_qkv[:], P_blk(k), w1_blk(k),
                             start=(k == 0), stop=(k == 3))

        # copy into the 32-col-block transpose layout (bf16); the Vv
        # columns (192:288, untransposed) follow separately off-path
        nc.scalar.copy(sb_t32[0:8, 0:192], ps_qkv[:, 0:192])

        sb_tT = sbt([32, 192], "sb_tT", bf16)
        nc.vector.transpose(sb_tT[:], sb_t32[:, 0:192])
        nc.scalar.copy(sb_t32[0:8, 192:288], ps_qkv[:, 192:288])

        def QkT_h(h):
            return sb_tT[0:32, 32 * h:32 * h + 8]

        def KkT_h(h):
            return sb_tT[0:32, 96 + 32 * h:96 + 32 * h + 8]

        def Vv_h(h):
            return sb_t32[0:8, 192 + 32 * h:192 + 32 * h + 32]

        # MHA1 scores, per head
        ps_s = pst([8, 24], "ps_s")
        for h in range(3):
            nc.tensor.matmul(ps_s[:, 8 * h:8 * h + 8], QkT_h(h), KkT_h(h))

        # softmax1 (no max-subtraction; magnitudes are small), normalized A
        # written into the 32x32-block layout
        a32v = sb_a32[0:8, :].rearrange("p (h x) -> p h x", h=3)[:, :, 0:8]
        nc.scalar.activation(a32v, ps_s[:].rearrange("p (h x) -> p h x", h=3),
                             ACT.Exp, scale=SCL)
        sb_sums = sbt([8, 3], "sb_sums")
        nc.vector.tensor_reduce(sb_sums[:], a32v, AX.X, ALU.add)
        sb_rec = sbt([8, 3], "sb_rec")
        nc.vector.reciprocal(sb_rec[:], sb_sums[:])
        rec_ap = sb_rec[:]
        rec_bc = bass.AP(tensor=rec_ap.tensor, offset=rec_ap.offset,
                         ap=[rec_ap.ap[0], rec_ap.ap[1], [0, 8]])
        nc.vector.tensor_tensor(a32v, a32v, rec_bc, ALU.mult)
        sb_aT32 = sbt([32, 96], "sb_aT32", bf16)
        nc.vector.transpose(sb_aT32[:], sb_a32[:])

        def A_T(h):
            return sb_aT32[0:8, 32 * h:32 * h + 8]

        # qT: 4 fp8 contraction chunks over Wqr (b landed by now)
        ps_qT = pst([96, 1], "ps_qT")
        for k in range(4):
            nc.tensor.matmul(ps_qT[:],
                             sb_b[:, B_WQR + 96 * k:B_WQR + 96 * k + 96],
                             xT_blk(k), start=(k == 0), stop=(k == 3))

        # alphaT (128,4): 16 fp8 (k,m) chunk matmuls; ba is folded in via a
        # host-precomputed exp(-ba) factor in the sigmoid denominator
        ps_al = pst([128, 4], "ps_al")

        def alpha_mms(ms):
            for m in ms:
                for k in range(4):
                    nc.tensor.matmul(
                        ps_al[:, m:m + 1],
                        sb_b[:, 512 * k + 128 * m:512 * k + 128 * m + 128],
                        xT_blk(k), start=(k == 0), stop=(k == 3))

        alpha_mms([0, 1])

        # O^T per head via tile_position: each head accumulates
        # Vv_h^T @ A_h^T plus Qk_h^T (via an identity-matmul) into its own
        # 32-row PSUM partition range -> H^T lands assembled, no copies
        ps_oT = pst([96, 8], "ps_oT")
        I8 = sb_c[0:8, C_I8:C_I8 + 8]
        for h in range(3):
            nc.tensor.matmul(ps_oT[32 * h:32 * h + 32, :], Vv_h(h), A_T(h),
                             start=True, stop=False, tile_position=(0, 32 * h))
            nc.tensor.matmul(ps_oT[32 * h:32 * h + 32, :],
                             sb_t32[0:8, 32 * h:32 * h + 32], I8,
                             start=False, stop=True, tile_position=(0, 32 * h))

        # qqT = wq2^T @ qT (qT relu'd on ACT during softmax)
        sb_qT = sbt([96, 1], "sb_qT", bf16)
        nc.scalar.activation(sb_qT[:], ps_qT[:], ACT.Relu,
                             bias=sb_f[0:96, F_BQR:F_BQR + 1])
        ps_qqT = pst([96, 1], "ps_qqT")
        nc.tensor.matmul(ps_qqT[:], sb_c[0:96, C_WQ2:C_WQ2 + 96], sb_qT[:])

        alpha_mms([2, 3])

        sb_hT = sbt([96, 8], "sb_hT", bf16)
        nc.vector.tensor_copy(sb_hT[:], ps_oT[:])

        # ---- fc1 residual: h2T = hT + relu(fc1^T @ hT), fused ----
        ps_rT = pst([96, 8], "ps_rT")
        nc.tensor.matmul(ps_rT[:], sb_c[0:96, C_F1:C_F1 + 96], sb_hT[:])
        sb_h2T = sbt([96, 8], "sb_h2T", bf16)
        nc.vector.scalar_tensor_tensor(sb_h2T[:], ps_rT[:], 0.0, sb_hT[:],
                                       ALU.max, ALU.add)

        # de/A precompute on gpsimd (only needs f): A = sum(De'9 * gate9)
        # with regs and the +offs fold baked in host-side
        sb_dp = sbt([128, 36], "sb_dp")
        de_v = sb_f[:, F_DE:F_DE + 36].rearrange("p (m s) -> p m s", m=4)
        g_ap = sb_f[:, F_G9:F_G9 + 9]
        g_bc = bass.AP(tensor=g_ap.tensor, offset=g_ap.offset,
                       ap=[g_ap.ap[0], [0, 4], g_ap.ap[1]])
        nc.gpsimd.tensor_tensor(sb_dp[:].rearrange("p (m s) -> p m s", m=4),
                                de_v, g_bc, ALU.mult)
        sb_A = sbt([128, 4], "sb_A")
        nc.vector.tensor_reduce(sb_A[:],
                                sb_dp[:].rearrange("p (m s) -> p m s", m=4),
                                AX.X, ALU.add)
        # de_r = A - offs  (F_NOF holds -offs)
        sb_der = sbt([128, 4], "sb_der")
        nc.gpsimd.tensor_scalar_add(sb_der[:], sb_A[:],
                                    sb_f[:, F_NOF:F_NOF + 1])

        # ---- stage 2 ----
        ps_k2T = pst([96, 8], "ps_k2T")
        nc.tensor.matmul(ps_k2T[:], sb_c[0:96, C_WK2:C_WK2 + 96], sb_h2T[:])
        ps_v2T = pst([96, 8], "ps_v2T")
        nc.tensor.matmul(ps_v2T[:], sb_c[0:96, C_WV2:C_WV2 + 96], sb_h2T[:])

        sb_qqT = sbt([96, 1], "sb_qqT")
        nc.vector.tensor_copy(sb_qqT[:], ps_qqT[:])
        sb_tmp = sbt([96, 8], "sb_tmp", bf16)
        nc.vector.tensor_scalar_mul(sb_tmp[:], ps_k2T[:], sb_qqT[:])

        ps_s2 = pst([16, 8], "ps_s2")
        nc.tensor.matmul(ps_s2[:], sb_c[0:96, C_E96:C_E96 + 16], sb_tmp[:])

        # alpha sigmoid tail: 1/(1 + exp(-z)*exp(-ba)) off PSUM; pushed late
        # in the static schedule (tile_wait_until) so it cannot head-of-line
        # block the softmax-critical DVE/ACT queues
        sb_en = sbt([128, 4], "sb_en")
        sb_dn = sbt([128, 4], "sb_dn")
        sb_alp = sbt([128, 4], "sb_alp")
        with tc.tile_wait_until(0.012):
            nc.scalar.activation(sb_en[:], ps_al[:], ACT.Exp, scale=-1.0)
            nc.gpsimd.tensor_tensor(sb_dn[:], sb_en[:],
                                    sb_f[:, F_ENB:F_ENB + 4], ALU.mult)
            nc.gpsimd.tensor_scalar_add(sb_dn[:], sb_dn[:], 1.0)
            nc.vector.reciprocal(sb_alp[:], sb_dn[:])

        sb_v2T = sbt([96, 8], "sb_v2T", bf16)
        nc.scalar.copy(sb_v2T[:], ps_v2T[:])

        # softmax2: exp with fused row-sum accumulator
        sb_e2 = sbt([16, 8], "sb_e2", bf16)
        sb_sum2 = sbt([16, 1], "sb_sum2")
        nc.scalar.activation(sb_e2[:], ps_s2[:], ACT.Exp, scale=SCL,
                             accum_out=sb_sum2[:])
        sb_rec2 = sbt([16, 1], "sb_rec2")
        nc.vector.reciprocal(sb_rec2[:], sb_sum2[:])
        sb_a2 = sbt([16, 8], "sb_a2", bf16)
        nc.vector.tensor_scalar_mul(sb_a2[:], sb_e2[:], sb_rec2[:])

        ps_a2e = pst([96, 8], "ps_a2e")
        nc.tensor.matmul(ps_a2e[:], sb_c[0:16, C_E2T:C_E2T + 96], sb_a2[:])

        sb_alr = sbt([128, 4], "sb_alr")
        sb_D = sbt([128, 4], "sb_D")
        sb_B = sbt([128, 4], "sb_B")
        with tc.tile_wait_until(0.013):
            nc.gpsimd.tensor_tensor(sb_alr[:], sb_alp[:],
                                    sb_f[:, F_RG:F_RG + 4], ALU.mult)
            nc.gpsimd.tensor_tensor(sb_D[:], sb_alp[:], sb_der[:], ALU.mult)
            nc.gpsimd.tensor_tensor(sb_B[:], sb_A[:], sb_D[:], ALU.subtract)

        # O2 = sum_h A2 * V2 (broadcast via E2T matmul)
        # (tensor_tensor_reduce crashes HW - NRT_EXEC_UNIT_UNRECOVERABLE)
        sb_scr = sbt([96, 8], "sb_scr")
        sb_o2T = sbt([96, 1], "sb_o2T")
        nc.vector.tensor_mul(sb_scr[:], ps_a2e[:], sb_v2T[:])
        nc.vector.tensor_reduce(sb_o2T[:], sb_scr[:], AX.X, ALU.add)
        sb_ot1 = sbt([96, 1], "sb_ot1", bf16)
        nc.vector.tensor_add(sb_ot1[:], sb_o2T[:], sb_qqT[:])

        # fc2 residual, fused relu+add
        ps_r2 = pst([96, 1], "ps_r2")
        nc.tensor.matmul(ps_r2[:], sb_c[0:96, C_F2:C_F2 + 96], sb_ot1[:])
        sb_otf = sbt([96, 1], "sb_otf", bf16)
        nc.vector.scalar_tensor_tensor(sb_otf[:], ps_r2[:], 0.0, sb_ot1[:],
                                       ALU.max, ALU.add)

        # ---- tail: transT then 2-op FiLM mix ----
        ps_tr = pst([128, 4], "ps_tr")
        for m in range(4):
            nc.tensor.matmul(ps_tr[:, m:m + 1],
                             sb_c[0:96, C_WO + 128 * m:C_WO + 128 * m + 128],
                             sb_otf[:])
        sb_t1 = sbt([128, 4], "sb_t1")
        nc.vector.tensor_mul(sb_t1[:], ps_tr[:], sb_alr[:])
        sb_o = sbt([128, 4], "sb_o")
        nc.vector.tensor_add(sb_o[:], sb_t1[:], sb_B[:])

        nc.scalar.dma_start(out=d_out[:], in_=sb_o[:])

    nc.compile()
    return nc


def _to_chunks128(a, cols):
    """(512, cols) -> (128, 4*cols) with column block k = rows [128k, 128k+128)."""
    return np.ascontiguousarray(
        a.reshape(4, 128, cols).transpose(1, 0, 2).reshape(128, 4 * cols))


def _pack_inputs(inputs):
    import ml_dtypes
    fp8 = ml_dtypes.float8_e4m3
    f32 = np.float32

    gate = np.asarray(inputs['gate'], f32)
    x = np.asarray(inputs['x'], f32)
    Wa = np.asarray(inputs['Wa'], f32)
    ba = np.asarray(inputs['ba'], f32)
    Wqr = np.asarray(inputs['Wqr'], f32)
    bqr = np.asarray(inputs['bqr'], f32)
    P = np.asarray(inputs['P'], f32)
    De = np.asarray(inputs['De'], f32)
    regs = np.asarray(inputs['regs'], f32)
    Wq1 = np.asarray(inputs['Wq1'], f32)
    Wk1 = np.asarray(inputs['Wk1'], f32)
    Wv1 = np.asarray(inputs['Wv1'], f32)
    fc1 = np.asarray(inputs['fc1'], f32)
    Wq2 = np.asarray(inputs['Wq2'], f32)
    Wk2 = np.asarray(inputs['Wk2'], f32)
    Wv2 = np.asarray(inputs['Wv2'], f32)
    fc2 = np.asarray(inputs['fc2'], f32)
    Wo = np.asarray(inputs['Wo'], f32)

    # b: wa chunks then wqr chunks (shared across cores)
    b_pack = np.concatenate([
        _to_chunks128(Wa, NM),
        _to_chunks128(Wqr, HID),
    ], axis=1).astype(fp8)
    b_pack = np.ascontiguousarray(b_pack)

    xT4 = np.ascontiguousarray(x.reshape(4, 128).T)
    baT4 = np.ascontiguousarray(ba.reshape(4, 128).T)
    g9 = np.concatenate([gate.reshape(1, 8), [[1.0]]], axis=1)
    g9_128 = np.ascontiguousarray(np.tile(g9, (128, 1)), f32)

    # E masks for 16 heads of dim 6
    idx = np.arange(HID) // 6
    E96 = np.zeros((96, 16), f32)
    E96[np.arange(96), idx] = 1.0
    E2T = np.ascontiguousarray(E96.T)         # (16, 96)

    in_maps = []
    for i in range(NPROC):
        bi, t = i // 4, i % 4
        offs = 1.0 if t in (0, 2) else 0.0

        w1 = np.concatenate(
            [np.concatenate([Wq1[i][128 * k:128 * k + 128],
                             Wk1[i][128 * k:128 * k + 128],
                             Wv1[i][128 * k:128 * k + 128]], axis=1)
             for k in range(4)], axis=1)
        a_pack = np.ascontiguousarray(np.concatenate([
            _to_chunks128(P[bi, t], SEQ), w1, xT4], axis=1).astype(fp8))

        c_np = np.zeros((128, C_COLS), f32)
        c_np[0:96, C_F1:C_F1 + 96] = fc1[i]
        c_np[0:96, C_WQ2:C_WQ2 + 96] = Wq2[i]
        c_np[0:96, C_WK2:C_WK2 + 96] = Wk2[i]
        c_np[0:96, C_WV2:C_WV2 + 96] = Wv2[i]
        c_np[0:96, C_F2:C_F2 + 96] = fc2[i]
        c_np[0:96, C_WO:C_WO + 512] = Wo[i]
        c_np[0:16, C_E2T:C_E2T + 96] = E2T
        c_np[0:96, C_E96:C_E96 + 16] = E96
        c_np[0:8, C_I8:C_I8 + 8] = np.eye(8, dtype=f32)
        c_pack = np.ascontiguousarray(c_np.astype(fp8))

        de9 = np.zeros((128, 4, 9), f32)
        de9[:, :, 0:8] = (De[bi, t] * regs[bi, t][:, None]).reshape(
            4, 128, SEQ).transpose(1, 0, 2)
        de9[:, :, 8] = offs
        f_np = np.zeros((128, F_COLS), f32)
        f_np[:, F_DE:F_DE + 36] = de9.reshape(128, 36)
        f_np[:, F_G9:F_G9 + 9] = g9_128
        f_np[:, F_ENB:F_ENB + 4] = np.exp(-baT4)
        f_np[0:96, F_BQR] = bqr
        f_np[:, F_NOF] = -offs
        f_np[:, F_RG:F_RG + 4] = regs[bi, t].reshape(4, 128).T

        in_maps.append({
            'a': a_pack,
            'b': b_pack,
            'c': c_pack,
            'f': np.ascontiguousarray(f_np),
        })
    return in_maps


def _run(inputs, trace=False):
    from concourse.bass_utils import run_bass_kernel_spmd
    if 'nc' not in _CACHE:
        _CACHE['nc'] = _build_nc()
    nc = _CACHE['nc']
    in_maps = _pack_inputs(inputs)
    res = run_bass_kernel_spmd(nc, in_maps, list(range(NPROC)), trace=trace)
    out = np.zeros((NB, 4, NM), np.float32)
    for i in range(NPROC):
        out[i // 4, i % 4] = np.asarray(res.results[i]['out']).T.reshape(NM)
    return out, res


def kernel(**inputs):
    out, _ = _run(inputs, trace=False)
    return out
